# revision 30
# baseline (speedup 1.0000x reference)
"""Bass/Trainium2 kernel for nn_BigramLanguageModel (6-layer GPT, E=768, H=6, T=1024, B=4, V=32000).

Sharding: 8-way token split (core c = batch c//2, sequence half c%2), full weights per
core in bf16. Per-layer K/V AllGather within core pairs, one 8-way x_f AllGather before
the vocab-sharded (4000/core) LM head, one tiny AllReduce for the CE partition function.

Activations are feature-major [feat_part, tok]. LayerNorm scales are folded into the
following matmul weights on the host (biases and LN biases in this problem are zero by
construction of setup_inputs; they are checked and must be zero). LN stats are computed
with ones-matmuls (float32r) + PE broadcast. Attention uses the S^T = K@Q^T layout so no
score transposes are needed; causal masking is a host-shipped multiplicative mask after
exp (scores are O(1), so no max-subtraction is needed); softmax denominators come from a
ones-matmul and exp(-ln(x)) reciprocals.
"""
from contextlib import ExitStack

import numpy as np
import ml_dtypes

import concourse.bass as bass
import concourse.tile as tile
import concourse.mybir as mybir
from concourse import bacc
from concourse.bass_utils import run_bass_kernel_spmd
from concourse.masks import make_identity

P = 128
E = 768
KO = E // P            # 6 feature tiles
T = 1024
TOK = 512              # tokens per core
H = 6
HD = 128
L = 6
V = 32000
VS = V // 8            # 4000 vocab per core
VSP = 4096             # padded
FF = 4 * E             # 3072
FKO = FF // P          # 24
HK = FKO // 2          # 12
NT = 8                 # global token chunks of 512
MV = VSP // P          # 32 vocab m-tiles per core
LAST_ROWS = VS - (MV - 1) * P  # 32 valid rows in last vocab tile
BF = mybir.dt.bfloat16
F32 = mybir.dt.float32
F32R = mybir.dt.float32r
BF16NP = ml_dtypes.bfloat16
AF = mybir.ActivationFunctionType
SUB = mybir.AluOpType.subtract

PAIRS = [[0, 1], [2, 3], [4, 5], [6, 7]]
ALL8 = [list(range(8))]

_CACHE = {}


class _Ctx:
    """Carrier for nc/tc/pools/consts shared across build helpers."""
    pass


def _mm(nc, out, lhsT, rhs, start, stop):
    nc.tensor.matmul(out, lhsT, rhs, start=start, stop=stop)


def _ln_rows(g, x_ap):
    """mean/rstd rows for feature-major x [P, KO, TOK] -> (rstd, rm) [1, TOK] sbuf f32.

    Stats sums run as bf16 ones-matmuls (PSUM accumulates f32), so x is rounded to
    bf16 for the stats only — same precision class as the bf16 matmuls elsewhere."""
    nc = g.nc
    xb = g.stat.tile([P, KO, TOK], BF, tag="statb", name="xb")
    nc.vector.tensor_copy(xb, x_ap)
    x2 = g.stat.tile([P, KO, TOK], BF, tag="statb", name="x2")
    nc.vector.tensor_mul(x2, xb, xb)
    s1 = g.ps_row.tile([1, TOK], F32, tag="ps_row", name="ps_s1")
    s2 = g.ps_row.tile([1, TOK], F32, tag="ps_row", name="ps_s2")
    for ko in range(KO):
        _mm(nc, s1, g.ones_b, xb[:, ko, :], ko == 0, ko == KO - 1)
    for ko in range(KO):
        _mm(nc, s2, g.ones_b, x2[:, ko, :], ko == 0, ko == KO - 1)
    r_mu = g.rows.tile([1, TOK], F32, tag="r_mu", name="r_mu")
    nc.scalar.mul(r_mu, s1, 1.0 / E)
    r_var = g.rows.tile([1, TOK], F32, tag="scrA", name="r_var")
    nc.scalar.mul(r_var, s2, 1.0 / E)
    r_mu2 = g.rows.tile([1, TOK], F32, tag="scrB", name="r_mu2")
    nc.vector.tensor_mul(r_mu2, r_mu, r_mu)
    nc.vector.tensor_tensor(r_var, r_var, r_mu2, SUB)   # in-place: var
    nc.scalar.activation(r_mu2, r_var, AF.Ln, bias=g.eps_r)  # reuse tile: ln(var+eps)
    r_rstd = g.rows.tile([1, TOK], F32, tag="r_rstd", name="r_rstd")
    nc.scalar.activation(r_rstd, r_mu2, AF.Exp, scale=-0.5)
    r_rm = g.rows.tile([1, TOK], F32, tag="r_rm", name="r_rm")
    nc.vector.tensor_mul(r_rm, r_rstd, r_mu)
    return r_rstd, r_rm


def _bcast(g, row):
    """[1, TOK] f32 sbuf -> [P, TOK] f32 psum (row rounded to bf16 for the PE pass)."""
    nc = g.nc
    row_b = g.rows.tile([1, TOK], BF, tag="rowb", name="row_b")
    nc.vector.tensor_copy(row_b, row)
    pb = g.ps_mm.tile([P, TOK], F32, name="ps")
    _mm(nc, pb, g.ones_col_b, row_b, True, True)
    return pb


def _norm_to(g, dst, x_ap):
    """dst (bf16 [P,KO,TOK]) = (x - mu) * rstd via rows + PE broadcast."""
    nc = g.nc
    r_rstd, r_rm = _ln_rows(g, x_ap)
    pb_rstd = _bcast(g, r_rstd)
    pb_rm = _bcast(g, r_rm)
    tmp = g.stat.tile([P, KO, TOK], F32, tag="stat12", name="nrm_tmp")
    nc.vector.tensor_mul(tmp, x_ap, pb_rstd[:, None, :].to_broadcast((P, KO, TOK)))
    nc.vector.tensor_tensor(dst, tmp, pb_rm[:, None, :].to_broadcast((P, KO, TOK)), SUB)


def _proj(g, w_dram, rhs, dst):
    """dst[:, mo, :] (bf16) = (W.T @ rhs) for W [P,KO,E]-tiled lhsT weights."""
    nc = g.nc
    wt = g.w768.tile([P, KO, E], BF, tag="w768", name="wt")
    nc.sync.dma_start(wt, w_dram[:, :, :])
    for mo in range(KO):
        pm = g.ps_mm.tile([P, TOK], F32, name="ps")
        for ko in range(KO):
            _mm(nc, pm, wt[:, ko, mo * P:(mo + 1) * P], rhs[:, ko, :], ko == 0, ko == KO - 1)
        nc.vector.tensor_copy(dst[:, mo, :], pm)


def _attention(g, qT, kT_all, kv_bout, oT):
    nc = g.nc
    for hp in (0, 2, 4):
        # stream this head pair's V^T from the gathered bounce, transpose to token-major
        v_pair = g.vtok.tile([P, 2, 2, TOK], BF, tag="v_pair", name="v_pair", bufs=2)
        for hh in range(2):
            for r in range(2):
                nc.sync.dma_start(v_pair[:, hh, r, :], kv_bout[r, :, 1, hp + hh, :])
        v_tok = g.vtok.tile([P, 2, NT, HD], BF, tag="v_tok", name="v_tok")
        for hh in range(2):
            for kt in range(NT):
                pt_ps = g.ps_mm.tile([P, P], BF, tag="ps", name="ps")
                nc.tensor.transpose(
                    pt_ps,
                    v_pair[:, hh, kt // 4, (kt % 4) * P:(kt % 4 + 1) * P],
                    g.ident)
                nc.vector.tensor_copy(v_tok[:, hh, kt, :], pt_ps)
        for hh in range(2):
            h_ = hp + hh
            po = g.ps_hold.tile([P, TOK], F32, name="ps_o")
            psum_s = g.ps_row.tile([1, TOK], F32, tag="ps_row", name="ps_den")
            for kt in range(NT):
                ps_s = g.ps_mm.tile([P, TOK], F32, name="ps")
                _mm(nc, ps_s,
                    kT_all[:, h_, kt // 4, (kt % 4) * P:(kt % 4 + 1) * P],
                    qT[:, h_, :], True, True)
                pt = g.ptp.tile([P, TOK], BF, name="pt")
                nc.scalar.activation(pt, ps_s, AF.Exp)
                nc.vector.tensor_mul(pt, pt, g.maskT[:, kt, :])
                _mm(nc, psum_s, g.ones_b, pt, kt == 0, kt == NT - 1)
                _mm(nc, po, v_tok[:, hh, kt, :], pt, kt == 0, kt == NT - 1)
            r_ln2 = g.rows.tile([1, TOK], F32, tag="scrA", name="r_ln2")
            nc.scalar.activation(r_ln2, psum_s, AF.Ln)
            r_rec = g.rows.tile([1, TOK], F32, tag="scrB", name="r_rec")
            nc.scalar.activation(r_rec, r_ln2, AF.Exp, scale=-1.0)
            pb = _bcast(g, r_rec)
            rb = g.stat.tile([P, TOK], F32, tag="rb", name="rb")
            nc.vector.tensor_copy(rb, pb)
            nc.vector.tensor_mul(oT[:, h_, :], po, rb)


def _layer(g, l):
    nc = g.nc
    x = g.x
    # ---- LN1 + QKV ----
    h = g.hpool.tile([P, KO, TOK], BF, name="h")
    _norm_to(g, h, x)
    qkvT = g.qkv.tile([P, 3, KO, TOK], BF, name="qkvT")
    _proj(g, g.wq_in[l], h, qkvT[:, 0])
    _proj(g, g.wk_in[l], h, qkvT[:, 1])
    _proj(g, g.wv_in[l], h, qkvT[:, 2])

    # ---- K/V pair AllGather ----
    kv_bin = g.dram.tile([P, 2, KO, TOK], BF, name="kv_bin")
    nc.sync.dma_start(kv_bin[:, 0], qkvT[:, 1])
    nc.sync.dma_start(kv_bin[:, 1], qkvT[:, 2])
    kv_bout = g.dram.tile([2, P, 2, KO, TOK], BF, name="kv_bout")
    nc.gpsimd.collective_compute(
        "AllGather", mybir.AluOpType.bypass, replica_groups=PAIRS,
        ins=[kv_bin[:].opt()], outs=[kv_bout[:].opt()])
    kT_all = g.kvag.tile([P, KO, 2, TOK], BF, name="kT_all")
    for r in range(2):
        nc.sync.dma_start(kT_all[:, :, r, :], kv_bout[r, :, 0])

    # ---- attention ----
    oT = g.otp.tile([P, KO, TOK], BF, name="oT")
    _attention(g, qkvT[:, 0], kT_all, kv_bout, oT)

    # ---- Wo + residual ----
    wt = g.w768.tile([P, KO, E], BF, tag="w768", name="wt")
    nc.sync.dma_start(wt, g.wo_in[l][:, :, :])
    for mo in range(KO):
        pm = g.ps_mm.tile([P, TOK], F32, name="ps")
        for ko in range(KO):
            _mm(nc, pm, wt[:, ko, mo * P:(mo + 1) * P], oT[:, ko, :], ko == 0, ko == KO - 1)
        nc.vector.tensor_add(x[:, mo, :], pm, x[:, mo, :])

    # ---- LN2 + FFN ----
    h2 = g.hpool.tile([P, KO, TOK], BF, name="h")
    _norm_to(g, h2, x)
    hid = g.hidp.tile([P, FKO, TOK], BF, name="hid")
    QM = KO  # m-tiles per w1 quarter
    for q in range(4):
        w1t = g.w768.tile([P, KO, E], BF, tag="w768", name="w1t")
        nc.sync.dma_start(w1t, g.w1_in[l][:, :, q * E:(q + 1) * E])
        for mo in range(QM):
            gmo = q * QM + mo
            pm = g.ps_mm.tile([P, TOK], F32, name="ps")
            for ko in range(KO):
                _mm(nc, pm, w1t[:, ko, mo * P:(mo + 1) * P], h2[:, ko, :], ko == 0, ko == KO - 1)
            nc.scalar.activation(hid[:, gmo, :], pm, AF.Relu)
    # w2: contract FF=3072 in quarters of 6 k-tiles; hold 3 output psums per mo-group
    for mog in (0, 3):
        pms = [g.ps_mm.tile([P, TOK], F32, name="ps") for _ in range(3)]
        for q in range(4):
            w2t = g.w768.tile([P, KO, E], BF, tag="w768", name="w2t")
            nc.sync.dma_start(w2t, g.w2_in[l][:, q * KO:(q + 1) * KO, :])
            for ko in range(KO):
                gko = q * KO + ko
                for mi in range(3):
                    mo = mog + mi
                    _mm(nc, pms[mi], w2t[:, ko, mo * P:(mo + 1) * P],
                        hid[:, gko, :], gko == 0, gko == FKO - 1)
        for mi in range(3):
            mo = mog + mi
            nc.vector.tensor_add(x[:, mo, :], pms[mi], x[:, mo, :])


def _blocks(g):
    """All transformer layers + final LN + x_f AllGather. Opens its own SBUF pools."""
    nc, tc = g.nc, g.tc
    with ExitStack() as es:
        g.rows = es.enter_context(tc.tile_pool(name="rows", bufs=1))
        g.stat = es.enter_context(tc.tile_pool(name="stat", bufs=2))
        g.hpool = es.enter_context(tc.tile_pool(name="hpool", bufs=1))
        g.qkv = es.enter_context(tc.tile_pool(name="qkv", bufs=1))
        g.kvag = es.enter_context(tc.tile_pool(name="kvag", bufs=1))
        g.vtok = es.enter_context(tc.tile_pool(name="vtok", bufs=1))
        g.ptp = es.enter_context(tc.tile_pool(name="ptp", bufs=3))
        g.otp = es.enter_context(tc.tile_pool(name="otp", bufs=1))
        g.hidp = es.enter_context(tc.tile_pool(name="hidp", bufs=1))
        g.w768 = es.enter_context(tc.tile_pool(name="w768", bufs=3))

        for l in range(L):
            _layer(g, l)

        xn = g.hpool.tile([P, KO, TOK], BF, name="h")
        _norm_to(g, xn, g.x)
        xn_bin = g.dram.tile([P, KO, TOK], BF, name="xn_bin")
        nc.sync.dma_start(xn_bin, xn)
        xn_bout = g.dram.tile([NT, P, KO, TOK], BF, name="xn_bout", addr_space="Shared")
        nc.gpsimd.collective_compute(
            "AllGather", mybir.AluOpType.bypass, replica_groups=ALL8,
            ins=[xn_bin[:].opt()], outs=[xn_bout[:].opt()])
        g.xn_bout = xn_bout


def _lm_head(g):
    nc, tc = g.nc, g.tc
    with ExitStack() as es:
        lmw = es.enter_context(tc.tile_pool(name="lmw", bufs=1))
        lmrow = es.enter_context(tc.tile_pool(name="lmrow", bufs=1))
        lmxn = es.enter_context(tc.tile_pool(name="lmxn", bufs=2))
        lmevac = es.enter_context(tc.tile_pool(name="lmevac", bufs=3))

        wlm_t = lmw.tile([P, KO, VSP], BF, name="wlm_t")
        nc.sync.dma_start(wlm_t, g.wlm_in[:, :, :])
        pick_row = lmrow.tile([1, NT, TOK], F32, name="pick_row")
        S_row = lmrow.tile([1, NT, TOK], F32, tag="lmrow16", name="S_row")

        for tc8 in range(NT):
            xn_c = lmxn.tile([P, KO, TOK], BF, tag="xn_c", name="xn_c")
            nc.sync.dma_start(xn_c, g.xn_bout[tc8])

            # target-pick partial (full row; identical on every core)
            wtg = lmxn.tile([P, KO, TOK], BF, tag="wtg", name="wtg")
            nc.sync.dma_start(wtg, g.wtgt_in[:, :, tc8, :])
            tmp = lmxn.tile([P, KO, TOK], BF, tag="pick_tmp", name="pick_tmp")
            nc.vector.tensor_mul(tmp, xn_c, wtg)
            pp = g.ps_row.tile([1, TOK], F32, tag="ps_row", name="pp")
            for ko in range(KO):
                _mm(nc, pp, g.ones_b, tmp[:, ko, :], ko == 0, ko == KO - 1)
            nc.scalar.copy(pick_row[:, tc8, :], pp)

            # logits + sumexp partials over this core's vocab shard
            pS = g.ps_row.tile([1, TOK], F32, tag="ps_row", name="pS")
            for mo in range(MV):
                pm = g.ps_mm.tile([P, TOK], F32, name="ps")
                for ko in range(KO):
                    _mm(nc, pm, wlm_t[:, ko, mo * P:(mo + 1) * P], xn_c[:, ko, :],
                        ko == 0, ko == KO - 1)
                et = lmevac.tile([P, TOK], BF, tag="et", name="et")
                nc.scalar.activation(et, pm, AF.Exp)
                lt = lmevac.tile([P, TOK], F32, tag="lt", name="lt")
                nc.vector.tensor_copy(lt, pm)
                nc.sync.dma_start(
                    g.logits_out[mo * P:(mo + 1) * P, tc8 * TOK:(tc8 + 1) * TOK], lt)
                kl = LAST_ROWS if mo == MV - 1 else P
                _mm(nc, pS, g.ones_b[:kl], et[:kl], mo == 0, mo == MV - 1)
            nc.scalar.copy(S_row[:, tc8, :], pS)

        # CE AllReduce of partition-function partials + loss
        ce_bin = g.dram.tile([1, NT * TOK], F32, name="ce_bin")
        nc.sync.dma_start(ce_bin, S_row.rearrange("o a b -> o (a b)"))
        ce_bout = g.dram.tile([1, NT * TOK], F32, name="ce_bout", addr_space="Shared")
        nc.gpsimd.collective_compute(
            "AllReduce", mybir.AluOpType.add, replica_groups=ALL8,
            ins=[ce_bin[:].opt()], outs=[ce_bout[:].opt()])
        S_all = lmrow.tile([1, NT * TOK], F32, tag="lmrow16", name="S_all")
        nc.sync.dma_start(S_all, ce_bout)
        lnS = lmrow.tile([1, NT * TOK], F32, name="lnS")
        nc.scalar.activation(lnS, S_all, AF.Ln)
        pr_flat = pick_row.rearrange("o a b -> o (a b)")
        nc.vector.tensor_tensor(pr_flat, pr_flat, lnS, SUB)
        lsum = lmrow.tile([1, 1], F32, name="lsum")
        nc.vector.reduce_sum(lsum, pr_flat, axis=mybir.AxisListType.X)
        lval = lmrow.tile([1, 1], F32, name="lval")
        nc.scalar.mul(lval, lsum, -1.0 / (NT * TOK))
        nc.sync.dma_start(g.loss_out[:, :], lval)


def build_program():
    if "nc" in _CACHE:
        return _CACHE["nc"]
    nc = bacc.Bacc("TRN2", target_bir_lowering=False, debug=False, num_devices=8)
    g = _Ctx()
    g.nc = nc

    # ---- I/O declarations (all in final SBUF layouts) ----
    g.x0_in = nc.dram_tensor("x0", [P, KO, TOK], F32, kind="ExternalInput")
    g.mask_in = nc.dram_tensor("maskT", [P, NT, TOK], BF, kind="ExternalInput")
    g.wq_in = [nc.dram_tensor(f"wq{l}", [P, KO, E], BF, kind="ExternalInput") for l in range(L)]
    g.wk_in = [nc.dram_tensor(f"wk{l}", [P, KO, E], BF, kind="ExternalInput") for l in range(L)]
    g.wv_in = [nc.dram_tensor(f"wv{l}", [P, KO, E], BF, kind="ExternalInput") for l in range(L)]
    g.wo_in = [nc.dram_tensor(f"wo{l}", [P, KO, E], BF, kind="ExternalInput") for l in range(L)]
    g.w1_in = [nc.dram_tensor(f"w1{l}", [P, KO, FF], BF, kind="ExternalInput") for l in range(L)]
    g.w2_in = [nc.dram_tensor(f"w2{l}", [P, FKO, E], BF, kind="ExternalInput") for l in range(L)]
    g.wlm_in = nc.dram_tensor("wlm", [P, KO, VSP], BF, kind="ExternalInput")
    g.wtgt_in = nc.dram_tensor("wtgt", [P, KO, NT, TOK], BF, kind="ExternalInput")
    g.logits_out = nc.dram_tensor("logits", [VSP, NT * TOK], F32, kind="ExternalOutput")
    g.loss_out = nc.dram_tensor("loss", [1, 1], F32, kind="ExternalOutput")

    with tile.TileContext(nc) as tc:
        g.tc = tc
        with ExitStack() as es:
            consts = es.enter_context(tc.tile_pool(name="consts", bufs=1))
            xres = es.enter_context(tc.tile_pool(name="xres", bufs=1))
            g.ps_mm = es.enter_context(tc.tile_pool(name="ps_mm", bufs=4, space="PSUM"))
            g.ps_hold = es.enter_context(tc.tile_pool(name="ps_hold", bufs=2, space="PSUM"))
            g.ps_row = es.enter_context(tc.tile_pool(name="ps_row", bufs=2, space="PSUM"))
            g.dram = es.enter_context(tc.tile_pool(name="dram", bufs=2, space="DRAM"))

            g.ones_col_b = consts.tile([1, P], BF, name="ones_col_b")
            nc.vector.memset(g.ones_col_b, 1.0)
            g.ones_b = consts.tile([P, 1], BF, name="ones_b")
            nc.vector.memset(g.ones_b, 1.0)
            g.ident = consts.tile([P, P], BF, name="ident")
            make_identity(nc, g.ident)
            g.eps_r = consts.tile([1, 1], F32, name="eps_r")
            nc.vector.memset(g.eps_r, 1e-5)
            g.maskT = consts.tile([P, NT, TOK], BF, name="maskTs")
            nc.sync.dma_start(g.maskT, g.mask_in[:, :, :])

            g.x = xres.tile([P, KO, TOK], F32, name="x")
            nc.sync.dma_start(g.x, g.x0_in[:, :, :])

            _blocks(g)
            _lm_head(g)

    nc.compile()
    _CACHE["nc"] = nc
    return nc


def _feat_tiles(w):
    """[K, M] -> [P, K//P, M] (partition-inner feature tiling)."""
    K, M = w.shape
    return np.ascontiguousarray(w.reshape(K // P, P, M).transpose(1, 0, 2))


def prep_in_maps(inp):
    """Host-side sharding/layout prep: full inputs dict -> per-core in_maps."""
    idx = inp["idx"].astype(np.int64)
    targets = inp["targets"].astype(np.int64)
    scale = 1.0 / np.sqrt(HD)

    # biases must be zero for this kernel (they are, by setup_inputs construction)
    for bname in ("bq", "bk", "bv", "bo", "b1", "b2", "ln1_b", "ln2_b"):
        assert np.all(inp[bname] == 0.0), f"nonzero bias {bname} unsupported"
    assert np.all(inp["lnf_b"] == 0.0) and np.all(inp["blm"] == 0.0)

    # host-side weight prep (shared across cores)
    x_full = inp["tok_emb"][idx] + inp["pos_emb"][None, :, :]  # [B, T, E] f32
    common = {}
    for l in range(L):
        s1 = inp["ln1_s"][l][:, None]
        common[f"wq{l}"] = _feat_tiles(s1 * inp["Wq"][l] * scale).astype(BF16NP)
        common[f"wk{l}"] = _feat_tiles(s1 * inp["Wk"][l]).astype(BF16NP)
        common[f"wv{l}"] = _feat_tiles(s1 * inp["Wv"][l]).astype(BF16NP)
        common[f"wo{l}"] = _feat_tiles(inp["Wo"][l]).astype(BF16NP)
        common[f"w1{l}"] = _feat_tiles(inp["ln2_s"][l][:, None] * inp["w1"][l]).astype(BF16NP)
        common[f"w2{l}"] = _feat_tiles(inp["w2"][l]).astype(BF16NP)
    wlm_f = inp["lnf_s"][:, None] * inp["Wlm"]  # [E, V] f32, lnf folded
    tflat = targets.reshape(-1)  # [4096]
    common["wtgt"] = np.ascontiguousarray(
        _feat_tiles(wlm_f[:, tflat]).astype(BF16NP).reshape(P, KO, NT, TOK))

    in_maps = []
    for c in range(8):
        b, half = c // 2, c % 2
        sl = slice(half * TOK, (half + 1) * TOK)
        x0 = np.ascontiguousarray(
            x_full[b, sl, :].T.reshape(KO, P, TOK).transpose(1, 0, 2)).astype(np.float32)
        # causal mask^T [k, q] for this core's query window
        q_glob = np.arange(half * TOK, (half + 1) * TOK)
        k_glob = np.arange(T)
        m = (k_glob[:, None] <= q_glob[None, :]).astype(np.float32)  # [1024, 512]
        maskT = np.ascontiguousarray(m.reshape(NT, P, TOK).transpose(1, 0, 2)).astype(BF16NP)
        wlm_shard = np.zeros((E, VSP), np.float32)
        wlm_shard[:, :VS] = wlm_f[:, c * VS:(c + 1) * VS]
        m_c = dict(common)
        m_c["x0"] = x0
        m_c["maskT"] = maskT
        m_c["wlm"] = _feat_tiles(wlm_shard).astype(BF16NP)
        in_maps.append(m_c)
    return in_maps


def kernel(**inputs):
    inp = {k: np.asarray(v) for k, v in inputs.items()}
    in_maps = prep_in_maps(inp)
    nc = build_program()
    res = run_bass_kernel_spmd(nc, in_maps, core_ids=list(range(8)))
    _CACHE["last_result"] = res

    logits = np.empty((NT * TOK, V), np.float32)
    for c in range(8):
        logits[:, c * VS:(c + 1) * VS] = res.results[c]["logits"][:VS, :].T
    loss = np.float32(res.results[0]["loss"][0, 0])
    return logits, loss


# revision 48
# speedup vs baseline: 1.9245x; 1.9245x over previous
"""Bass/Trainium2 kernel for nn_BigramLanguageModel (6-layer GPT, E=768, H=6, T=1024, B=4, V=32000).

Sharding: 8-way token split (core c = batch c//2, sequence half c%2), full weights per
core in bf16. Per-layer K/V AllGather within core pairs, one 8-way x_f AllGather before
the vocab-sharded (4000/core) LM head, one tiny AllReduce for the CE partition function.

Activations are feature-major [feat_part, tok]. LayerNorm scales are folded into the
following matmul weights on the host (biases and LN biases in this problem are zero by
construction of setup_inputs; they are checked and must be zero). LN stats are computed
with ones-matmuls (float32r) + PE broadcast. Attention uses the S^T = K@Q^T layout so no
score transposes are needed; causal masking is a host-shipped multiplicative mask after
exp (scores are O(1), so no max-subtraction is needed); softmax denominators come from a
ones-matmul and exp(-ln(x)) reciprocals.
"""
from contextlib import ExitStack

import numpy as np
import ml_dtypes

import concourse.bass as bass
import concourse.tile as tile
import concourse.mybir as mybir
from concourse import bacc
from concourse.bass_utils import run_bass_kernel_spmd
from concourse.masks import make_identity

P = 128
E = 768
KO = E // P            # 6 feature tiles
T = 1024
TOK = 512              # tokens per core
H = 6
HD = 128
L = 6
V = 32000
VS = V // 8            # 4000 vocab per core
VSP = 4096             # padded
FF = 4 * E             # 3072
FKO = FF // P          # 24
HK = FKO // 2          # 12
NT = 8                 # global token chunks of 512
MV = VSP // P          # 32 vocab m-tiles per core
LAST_ROWS = VS - (MV - 1) * P  # 32 valid rows in last vocab tile
BF = mybir.dt.bfloat16
F32 = mybir.dt.float32
F32R = mybir.dt.float32r
BF16NP = ml_dtypes.bfloat16
AF = mybir.ActivationFunctionType
SUB = mybir.AluOpType.subtract

PAIRS = [[0, 1], [2, 3], [4, 5], [6, 7]]
ALL8 = [list(range(8))]

_CACHE = {}
SINGLE_CORE_SIM = False
ABLATE_LAYERS = False   # skip transformer layers (debug/bench only)
ABLATE_LM = False       # skip lm head (debug/bench only)


def _shared():
    return "Local" if SINGLE_CORE_SIM else "Shared"  # replace collectives with local DMA stand-ins (cost-model sim only)


def _collective(nc, kind, op, groups, ins, outs):
    if not SINGLE_CORE_SIM:
        nc.gpsimd.collective_compute(kind, op, replica_groups=groups,
                                     ins=[ins[0].opt()], outs=[outs[0].opt()])
        return
    # stand-in with equivalent DRAM traffic: replicate own buffer into each rank slot
    in_ap, out_ap = ins[0], outs[0]
    n = len(groups[0])
    per = out_ap.shape[0] // n if kind == "AllGather" else None
    if kind == "AllGather":
        for r in range(n):
            nc.sync.dma_start(out_ap[r * per:(r + 1) * per], in_ap)
    else:  # AllReduce
        nc.sync.dma_start(out_ap, in_ap)


def _setup_act_override():
    """Point walrus at an act_info.json with natural_log_exp_and_others first, so Exp and
    Ln resolve to ONE table set (first-fit selection otherwise splits them across two sets,
    costing a ~2.7us ACT table reload on every Ln<->Exp transition ~100x per run)."""
    import os, json, glob, tempfile
    if os.environ.get("BASS_ACT_ROOT_JSON_PATH"):
        return True
    try:
        import neuronxcc
        pwp = None
        for cand in glob.glob(os.path.join(os.path.dirname(neuronxcc.__file__),
                                           "pwp", "pwp_bin_*", "act_info.json")):
            pwp = cand
            break
        if pwp is None:
            return
        src_dir = os.path.dirname(pwp)
        d = json.load(open(pwp))
        sets = d.get("act_func_sets", [])
        pref = [s for s in sets if s.get("name") == "natural_log_exp_and_others"]
        if not pref:
            return False
        # offer ONLY the superset so every ACTIVATE resolves to one table set
        d["act_func_sets"] = pref
        dst = tempfile.mkdtemp(prefix="act_override_")
        for f in os.listdir(src_dir):
            try:
                os.symlink(os.path.join(src_dir, f), os.path.join(dst, f))
            except OSError:
                pass
        os.unlink(os.path.join(dst, "act_info.json"))
        with open(os.path.join(dst, "act_info.json"), "w") as fh:
            json.dump(d, fh)
        os.environ["BASS_ACT_ROOT_JSON_PATH"] = os.path.join(dst, "act_info.json")
        return True
    except Exception:
        return False  # fall back to stock tables (correct, just slower)


class _Ctx:
    """Carrier for nc/tc/pools/consts shared across build helpers."""
    pass


def _mm(nc, out, lhsT, rhs, start, stop):
    nc.tensor.matmul(out, lhsT, rhs, start=start, stop=stop)


def _ln_rows(g, x_ap):
    """mean/rstd rows for feature-major x [P, KO, TOK] -> (rstd, rm) [1, TOK] sbuf f32.

    Stats sums run as bf16 ones-matmuls (PSUM accumulates f32), so x is rounded to
    bf16 for the stats only — same precision class as the bf16 matmuls elsewhere."""
    nc = g.nc
    xb = g.stat.tile([P, KO, TOK], BF, tag="statb", name="xb")
    nc.vector.tensor_copy(xb, x_ap)
    x2 = g.stat.tile([P, KO, TOK], BF, tag="statb", name="x2")
    nc.vector.tensor_mul(x2, xb, xb)
    s1 = g.ps_row.tile([1, TOK], F32, tag="ps_row", name="ps_s1")
    s2 = g.ps_row.tile([1, TOK], F32, tag="ps_row", name="ps_s2")
    for ko in range(KO):
        _mm(nc, s1, g.ones_b, xb[:, ko, :], ko == 0, ko == KO - 1)
    for ko in range(KO):
        _mm(nc, s2, g.ones_b, x2[:, ko, :], ko == 0, ko == KO - 1)
    r_mu = g.rows.tile([1, TOK], F32, tag="r_mu", name="r_mu")
    nc.scalar.mul(r_mu, s1, 1.0 / E)
    r_var = g.rows.tile([1, TOK], F32, tag="scrA", name="r_var")
    nc.scalar.mul(r_var, s2, 1.0 / E)
    r_mu2 = g.rows.tile([1, TOK], F32, tag="scrB", name="r_mu2")
    nc.vector.tensor_mul(r_mu2, r_mu, r_mu)
    nc.vector.tensor_tensor(r_var, r_var, r_mu2, SUB)   # in-place: var
    nc.scalar.activation(r_mu2, r_var, AF.Ln, bias=g.eps_r)  # reuse tile: ln(var+eps)
    r_rstd = g.rows.tile([1, TOK], F32, tag="r_rstd", name="r_rstd")
    nc.scalar.activation(r_rstd, r_mu2, AF.Exp, scale=-0.5)
    r_rm = g.rows.tile([1, TOK], F32, tag="r_rm", name="r_rm")
    nc.vector.tensor_mul(r_rm, r_rstd, r_mu)
    return r_rstd, r_rm


def _bcast(g, row):
    """[1, TOK] f32 sbuf -> [P, TOK] f32 psum (row rounded to bf16 for the PE pass)."""
    nc = g.nc
    row_b = g.rows.tile([1, TOK], BF, tag="rowb", name="row_b")
    nc.vector.tensor_copy(row_b, row)
    pb = g.ps_mm.tile([P, TOK], F32, name="ps")
    _mm(nc, pb, g.ones_col_b, row_b, True, True)
    return pb


def _norm_to(g, dst, x_ap):
    """dst (bf16 [P,KO,TOK]) = (x - mu) * rstd via rows + PE broadcast."""
    nc = g.nc
    r_rstd, r_rm = _ln_rows(g, x_ap)
    pb_rstd = _bcast(g, r_rstd)
    pb_rm = _bcast(g, r_rm)
    tmp = g.stat.tile([P, KO, TOK], F32, tag="stat12", name="nrm_tmp")
    nc.vector.tensor_mul(tmp, x_ap, pb_rstd[:, None, :].to_broadcast((P, KO, TOK)))
    nc.vector.tensor_tensor(dst, tmp, pb_rm[:, None, :].to_broadcast((P, KO, TOK)), SUB)


def _proj(g, w_dram, rhs, dst):
    """dst[:, mo, :] (bf16) = (W.T @ rhs) for W [P,KO,E]-tiled lhsT weights."""
    nc = g.nc
    wt = g.w768.tile([P, KO, E], BF, tag="w768", name="wt")
    nc.sync.dma_start(wt, w_dram[:, :, :])
    for mo in range(KO):
        pm = g.ps_mm.tile([P, TOK], F32, name="ps")
        for ko in range(KO):
            _mm(nc, pm, wt[:, ko, mo * P:(mo + 1) * P], rhs[:, ko, :], ko == 0, ko == KO - 1)
        nc.vector.tensor_copy(dst[:, mo, :], pm)


def _attention(g, qT, kT_all, kv_bout, oT):
    nc = g.nc
    for hp in (0, 2, 4):
        # stream this head pair's V^T from the gathered bounce, transpose to token-major
        v_pair = g.vtok.tile([P, 2, 2, TOK], BF, tag="v_pair", name="v_pair", bufs=2)
        for hh in range(2):
            for r in range(2):
                nc.sync.dma_start(v_pair[:, hh, r, :], kv_bout[r, :, 1, hp + hh, :])
        v_tok = g.vtok.tile([P, 2, NT, HD], BF, tag="v_tok", name="v_tok")
        for hh in range(2):
            for kt in range(NT):
                pt_ps = g.ps_mm.tile([P, P], BF, tag="ps", name="ps")
                nc.tensor.transpose(
                    pt_ps,
                    v_pair[:, hh, kt // 4, (kt % 4) * P:(kt % 4 + 1) * P],
                    g.ident)
                nc.vector.tensor_copy(v_tok[:, hh, kt, :], pt_ps)
        for hh in range(2):
            h_ = hp + hh
            po = g.ps_hold.tile([P, TOK], F32, name="ps_o")
            psum_s = g.ps_row.tile([1, TOK], F32, tag="ps_row", name="ps_den")
            for kt in range(NT):
                ps_s = g.ps_mm.tile([P, TOK], F32, name="ps")
                _mm(nc, ps_s,
                    kT_all[:, h_, kt // 4, (kt % 4) * P:(kt % 4 + 1) * P],
                    qT[:, h_, :], True, True)
                pt = g.ptp.tile([P, TOK], BF, name="pt")
                nc.scalar.activation(pt, ps_s, AF.Exp)
                nc.vector.tensor_mul(pt, pt, g.maskT[:, kt, :])
                _mm(nc, psum_s, g.ones_b, pt, kt == 0, kt == NT - 1)
                _mm(nc, po, v_tok[:, hh, kt, :], pt, kt == 0, kt == NT - 1)
            r_ln2 = g.rows.tile([1, TOK], F32, tag="scrA", name="r_ln2")
            nc.scalar.activation(r_ln2, psum_s, AF.Ln)
            r_rec = g.rows.tile([1, TOK], F32, tag="scrB", name="r_rec")
            nc.scalar.activation(r_rec, r_ln2, AF.Exp, scale=-1.0)
            pb = _bcast(g, r_rec)
            rb = g.stat.tile([P, TOK], F32, tag="rb", name="rb")
            nc.vector.tensor_copy(rb, pb)
            nc.vector.tensor_mul(oT[:, h_, :], po, rb)


def _layer(g, l):
    nc = g.nc
    x = g.x
    # ---- LN1 + QKV ----
    h = g.hpool.tile([P, KO, TOK], BF, name="h")
    _norm_to(g, h, x)
    qkvT = g.qkv.tile([P, 3, KO, TOK], BF, name="qkvT")
    _proj(g, g.wq_in[l], h, qkvT[:, 0])
    _proj(g, g.wk_in[l], h, qkvT[:, 1])
    _proj(g, g.wv_in[l], h, qkvT[:, 2])

    # ---- K/V pair AllGather ----
    kv_bin = g.dram.tile([P, 2, KO, TOK], BF, name="kv_bin")
    nc.sync.dma_start(kv_bin[:, 0], qkvT[:, 1])
    nc.sync.dma_start(kv_bin[:, 1], qkvT[:, 2])
    kv_bout = g.dram.tile([2, P, 2, KO, TOK], BF, name="kv_bout")
    _collective(nc, "AllGather", mybir.AluOpType.bypass, PAIRS, [kv_bin[:]], [kv_bout[:]])
    kT_all = g.kvag.tile([P, KO, 2, TOK], BF, name="kT_all")
    for r in range(2):
        nc.sync.dma_start(kT_all[:, :, r, :], kv_bout[r, :, 0])

    # ---- attention ----
    oT = g.otp.tile([P, KO, TOK], BF, name="oT")
    _attention(g, qkvT[:, 0], kT_all, kv_bout, oT)

    # ---- Wo + residual ----
    wt = g.w768.tile([P, KO, E], BF, tag="w768", name="wt")
    nc.sync.dma_start(wt, g.wo_in[l][:, :, :])
    for mo in range(KO):
        pm = g.ps_mm.tile([P, TOK], F32, name="ps")
        for ko in range(KO):
            _mm(nc, pm, wt[:, ko, mo * P:(mo + 1) * P], oT[:, ko, :], ko == 0, ko == KO - 1)
        nc.vector.tensor_add(x[:, mo, :], pm, x[:, mo, :])

    # ---- LN2 + FFN ----
    h2 = g.hpool.tile([P, KO, TOK], BF, name="h")
    _norm_to(g, h2, x)
    hid = g.hidp.tile([P, FKO, TOK], BF, name="hid")
    QM = KO  # m-tiles per w1 quarter
    for q in range(4):
        w1t = g.w768.tile([P, KO, E], BF, tag="w768", name="w1t")
        nc.sync.dma_start(w1t, g.w1_in[l][:, :, q * E:(q + 1) * E])
        for mo in range(QM):
            gmo = q * QM + mo
            pm = g.ps_mm.tile([P, TOK], F32, name="ps")
            for ko in range(KO):
                _mm(nc, pm, w1t[:, ko, mo * P:(mo + 1) * P], h2[:, ko, :], ko == 0, ko == KO - 1)
            nc.scalar.activation(hid[:, gmo, :], pm, AF.Relu)
    # w2: contract FF=3072 in quarters of 6 k-tiles; hold 3 output psums per mo-group
    for mog in (0, 3):
        pms = [g.ps_mm.tile([P, TOK], F32, name="ps") for _ in range(3)]
        for q in range(4):
            w2t = g.w768.tile([P, KO, E], BF, tag="w768", name="w2t")
            nc.sync.dma_start(w2t, g.w2_in[l][:, q * KO:(q + 1) * KO, :])
            for ko in range(KO):
                gko = q * KO + ko
                for mi in range(3):
                    mo = mog + mi
                    _mm(nc, pms[mi], w2t[:, ko, mo * P:(mo + 1) * P],
                        hid[:, gko, :], gko == 0, gko == FKO - 1)
        for mi in range(3):
            mo = mog + mi
            nc.vector.tensor_add(x[:, mo, :], pms[mi], x[:, mo, :])


def _blocks(g):
    """All transformer layers + final LN + x_f AllGather. Opens its own SBUF pools."""
    nc, tc = g.nc, g.tc
    with ExitStack() as es:
        g.rows = es.enter_context(tc.tile_pool(name="rows", bufs=1))
        g.stat = es.enter_context(tc.tile_pool(name="stat", bufs=2))
        g.hpool = es.enter_context(tc.tile_pool(name="hpool", bufs=1))
        g.qkv = es.enter_context(tc.tile_pool(name="qkv", bufs=1))
        g.kvag = es.enter_context(tc.tile_pool(name="kvag", bufs=1))
        g.vtok = es.enter_context(tc.tile_pool(name="vtok", bufs=1))
        g.ptp = es.enter_context(tc.tile_pool(name="ptp", bufs=3))
        g.otp = es.enter_context(tc.tile_pool(name="otp", bufs=1))
        g.hidp = es.enter_context(tc.tile_pool(name="hidp", bufs=1))
        g.w768 = es.enter_context(tc.tile_pool(name="w768", bufs=3))

        for l in range(L):
            if not ABLATE_LAYERS:
                _layer(g, l)

        xn = g.hpool.tile([P, KO, TOK], BF, name="h")
        _norm_to(g, xn, g.x)
        xn_bin = g.dram.tile([P, KO, TOK], BF, name="xn_bin")
        nc.sync.dma_start(xn_bin, xn)
        xn_bout = g.dram.tile([NT, P, KO, TOK], BF, name="xn_bout", addr_space=_shared())
        _collective(nc, "AllGather", mybir.AluOpType.bypass, ALL8, [xn_bin[:]], [xn_bout[:]])
        g.xn_bout = xn_bout


def _lm_head(g):
    nc, tc = g.nc, g.tc
    with ExitStack() as es:
        lmw = es.enter_context(tc.tile_pool(name="lmw", bufs=1))
        lmrow = es.enter_context(tc.tile_pool(name="lmrow", bufs=1))
        lmxn = es.enter_context(tc.tile_pool(name="lmxn", bufs=2))
        lmevac = es.enter_context(tc.tile_pool(name="lmevac", bufs=3))

        wlm_t = lmw.tile([P, KO, VSP], BF, name="wlm_t")
        nc.sync.dma_start(wlm_t, g.wlm_in[:, :, :])
        pick_row = lmrow.tile([1, NT, TOK], F32, name="pick_row")
        S_row = lmrow.tile([1, NT, TOK], F32, tag="lmrow16", name="S_row")

        for tc8 in range(NT):
            xn_c = lmxn.tile([P, KO, TOK], BF, tag="xn_c", name="xn_c")
            nc.sync.dma_start(xn_c, g.xn_bout[tc8])

            # target-pick partial (full row; identical on every core)
            wtg = lmxn.tile([P, KO, TOK], BF, tag="wtg", name="wtg")
            nc.sync.dma_start(wtg, g.wtgt_in[:, :, tc8, :])
            tmp = lmxn.tile([P, KO, TOK], BF, tag="pick_tmp", name="pick_tmp")
            nc.vector.tensor_mul(tmp, xn_c, wtg)
            pp = g.ps_row.tile([1, TOK], F32, tag="ps_row", name="pp")
            for ko in range(KO):
                _mm(nc, pp, g.ones_b, tmp[:, ko, :], ko == 0, ko == KO - 1)
            nc.scalar.copy(pick_row[:, tc8, :], pp)

            # logits + sumexp partials over this core's vocab shard
            pS = g.ps_row.tile([1, TOK], F32, tag="ps_row", name="pS")
            for mo in range(MV):
                pm = g.ps_mm.tile([P, TOK], F32, name="ps")
                for ko in range(KO):
                    _mm(nc, pm, wlm_t[:, ko, mo * P:(mo + 1) * P], xn_c[:, ko, :],
                        ko == 0, ko == KO - 1)
                et = lmevac.tile([P, TOK], BF, tag="et", name="et")
                nc.scalar.activation(et, pm, AF.Exp)
                lt = lmevac.tile([P, TOK], F32, tag="lt", name="lt")
                nc.vector.tensor_copy(lt, pm)
                nc.sync.dma_start(
                    g.logits_out[mo * P:(mo + 1) * P, tc8 * TOK:(tc8 + 1) * TOK], lt)
                kl = LAST_ROWS if mo == MV - 1 else P
                _mm(nc, pS, g.ones_b[:kl], et[:kl], mo == 0, mo == MV - 1)
            nc.scalar.copy(S_row[:, tc8, :], pS)

        # CE AllReduce of partition-function partials + loss
        ce_bin = g.dram.tile([1, NT * TOK], F32, name="ce_bin")
        nc.sync.dma_start(ce_bin, S_row.rearrange("o a b -> o (a b)"))
        ce_bout = g.dram.tile([1, NT * TOK], F32, name="ce_bout", addr_space=_shared())
        _collective(nc, "AllReduce", mybir.AluOpType.add, ALL8, [ce_bin[:]], [ce_bout[:]])
        S_all = lmrow.tile([1, NT * TOK], F32, tag="lmrow16", name="S_all")
        nc.sync.dma_start(S_all, ce_bout)
        lnS = lmrow.tile([1, NT * TOK], F32, name="lnS")
        nc.scalar.activation(lnS, S_all, AF.Ln)
        pr_flat = pick_row.rearrange("o a b -> o (a b)")
        nc.vector.tensor_tensor(pr_flat, pr_flat, lnS, SUB)
        lsum = lmrow.tile([1, 1], F32, name="lsum")
        nc.vector.reduce_sum(lsum, pr_flat, axis=mybir.AxisListType.X)
        lval = lmrow.tile([1, 1], F32, name="lval")
        nc.scalar.mul(lval, lsum, -1.0 / (NT * TOK))
        nc.sync.dma_start(g.loss_out[:, :], lval)


def _patch_act_tables():
    """Restrict bacc's ACT-table view to natural_log_exp_and_others (index 0, matching the
    walrus act_info override) so all ACTIVATEs share one table set -> one ACT_TABLE_LOAD."""
    import concourse.hw_specs as hw_specs
    orig = hw_specs.get_activation_tables
    full = orig("gen3")
    name = "natural_log_exp_and_others"
    if name not in full:
        return None
    only = {name: full[name]}

    def patched(module_arch):
        return only

    hw_specs.get_activation_tables = patched
    bacc.get_activation_tables = patched

    def restore():
        hw_specs.get_activation_tables = orig
        bacc.get_activation_tables = orig
    return restore


def build_program():
    if "nc" in _CACHE:
        return _CACHE["nc"]
    _restore = _patch_act_tables() if _setup_act_override() else None
    nc = bacc.Bacc("TRN2", target_bir_lowering=False, debug=False, num_devices=8)
    g = _Ctx()
    g.nc = nc

    # ---- I/O declarations (all in final SBUF layouts) ----
    g.x0_in = nc.dram_tensor("x0", [P, KO, TOK], F32, kind="ExternalInput")
    g.mask_in = nc.dram_tensor("maskT", [P, NT, TOK], BF, kind="ExternalInput")
    g.wq_in = [nc.dram_tensor(f"wq{l}", [P, KO, E], BF, kind="ExternalInput") for l in range(L)]
    g.wk_in = [nc.dram_tensor(f"wk{l}", [P, KO, E], BF, kind="ExternalInput") for l in range(L)]
    g.wv_in = [nc.dram_tensor(f"wv{l}", [P, KO, E], BF, kind="ExternalInput") for l in range(L)]
    g.wo_in = [nc.dram_tensor(f"wo{l}", [P, KO, E], BF, kind="ExternalInput") for l in range(L)]
    g.w1_in = [nc.dram_tensor(f"w1{l}", [P, KO, FF], BF, kind="ExternalInput") for l in range(L)]
    g.w2_in = [nc.dram_tensor(f"w2{l}", [P, FKO, E], BF, kind="ExternalInput") for l in range(L)]
    g.wlm_in = nc.dram_tensor("wlm", [P, KO, VSP], BF, kind="ExternalInput")
    g.wtgt_in = nc.dram_tensor("wtgt", [P, KO, NT, TOK], BF, kind="ExternalInput")
    g.logits_out = nc.dram_tensor("logits", [VSP, NT * TOK], F32, kind="ExternalOutput")
    g.loss_out = nc.dram_tensor("loss", [1, 1], F32, kind="ExternalOutput")

    with tile.TileContext(nc) as tc:
        g.tc = tc
        with ExitStack() as es:
            consts = es.enter_context(tc.tile_pool(name="consts", bufs=1))
            xres = es.enter_context(tc.tile_pool(name="xres", bufs=1))
            g.ps_mm = es.enter_context(tc.tile_pool(name="ps_mm", bufs=4, space="PSUM"))
            g.ps_hold = es.enter_context(tc.tile_pool(name="ps_hold", bufs=2, space="PSUM"))
            g.ps_row = es.enter_context(tc.tile_pool(name="ps_row", bufs=2, space="PSUM"))
            g.dram = es.enter_context(tc.tile_pool(name="dram", bufs=2, space="DRAM"))

            g.ones_col_b = consts.tile([1, P], BF, name="ones_col_b")
            nc.vector.memset(g.ones_col_b, 1.0)
            g.ones_b = consts.tile([P, 1], BF, name="ones_b")
            nc.vector.memset(g.ones_b, 1.0)
            g.ident = consts.tile([P, P], BF, name="ident")
            make_identity(nc, g.ident)
            g.eps_r = consts.tile([1, 1], F32, name="eps_r")
            nc.vector.memset(g.eps_r, 1e-5)
            g.maskT = consts.tile([P, NT, TOK], BF, name="maskTs")
            nc.sync.dma_start(g.maskT, g.mask_in[:, :, :])

            g.x = xres.tile([P, KO, TOK], F32, name="x")
            nc.sync.dma_start(g.x, g.x0_in[:, :, :])

            _blocks(g)
            if ABLATE_LM:
                lval0 = xres.tile([1, 1], F32, name="lval0")
                nc.vector.memset(lval0, 0.0)
                nc.sync.dma_start(g.loss_out[:, :], lval0)
            else:
                _lm_head(g)

    try:
        nc.compile()
    finally:
        if _restore is not None:
            _restore()
    _CACHE["nc"] = nc
    return nc


def _feat_tiles(w):
    """[K, M] -> [P, K//P, M] (partition-inner feature tiling)."""
    K, M = w.shape
    return np.ascontiguousarray(w.reshape(K // P, P, M).transpose(1, 0, 2))


def prep_in_maps(inp):
    """Host-side sharding/layout prep: full inputs dict -> per-core in_maps."""
    idx = inp["idx"].astype(np.int64)
    targets = inp["targets"].astype(np.int64)
    scale = 1.0 / np.sqrt(HD)

    # biases must be zero for this kernel (they are, by setup_inputs construction)
    for bname in ("bq", "bk", "bv", "bo", "b1", "b2", "ln1_b", "ln2_b"):
        assert np.all(inp[bname] == 0.0), f"nonzero bias {bname} unsupported"
    assert np.all(inp["lnf_b"] == 0.0) and np.all(inp["blm"] == 0.0)

    # host-side weight prep (shared across cores)
    x_full = inp["tok_emb"][idx] + inp["pos_emb"][None, :, :]  # [B, T, E] f32
    common = {}
    for l in range(L):
        s1 = inp["ln1_s"][l][:, None]
        common[f"wq{l}"] = _feat_tiles(s1 * inp["Wq"][l] * scale).astype(BF16NP)
        common[f"wk{l}"] = _feat_tiles(s1 * inp["Wk"][l]).astype(BF16NP)
        common[f"wv{l}"] = _feat_tiles(s1 * inp["Wv"][l]).astype(BF16NP)
        common[f"wo{l}"] = _feat_tiles(inp["Wo"][l]).astype(BF16NP)
        common[f"w1{l}"] = _feat_tiles(inp["ln2_s"][l][:, None] * inp["w1"][l]).astype(BF16NP)
        common[f"w2{l}"] = _feat_tiles(inp["w2"][l]).astype(BF16NP)
    wlm_f = inp["lnf_s"][:, None] * inp["Wlm"]  # [E, V] f32, lnf folded
    tflat = targets.reshape(-1)  # [4096]
    common["wtgt"] = np.ascontiguousarray(
        _feat_tiles(wlm_f[:, tflat]).astype(BF16NP).reshape(P, KO, NT, TOK))

    in_maps = []
    for c in range(8):
        b, half = c // 2, c % 2
        sl = slice(half * TOK, (half + 1) * TOK)
        x0 = np.ascontiguousarray(
            x_full[b, sl, :].T.reshape(KO, P, TOK).transpose(1, 0, 2)).astype(np.float32)
        # causal mask^T [k, q] for this core's query window
        q_glob = np.arange(half * TOK, (half + 1) * TOK)
        k_glob = np.arange(T)
        m = (k_glob[:, None] <= q_glob[None, :]).astype(np.float32)  # [1024, 512]
        maskT = np.ascontiguousarray(m.reshape(NT, P, TOK).transpose(1, 0, 2)).astype(BF16NP)
        wlm_shard = np.zeros((E, VSP), np.float32)
        wlm_shard[:, :VS] = wlm_f[:, c * VS:(c + 1) * VS]
        m_c = dict(common)
        m_c["x0"] = x0
        m_c["maskT"] = maskT
        m_c["wlm"] = _feat_tiles(wlm_shard).astype(BF16NP)
        in_maps.append(m_c)
    return in_maps


def kernel(**inputs):
    inp = {k: np.asarray(v) for k, v in inputs.items()}
    in_maps = prep_in_maps(inp)
    nc = build_program()
    res = run_bass_kernel_spmd(nc, in_maps, core_ids=list(range(8)))
    _CACHE["last_result"] = res

    logits = np.empty((NT * TOK, V), np.float32)
    for c in range(8):
        logits[:, c * VS:(c + 1) * VS] = res.results[c]["logits"][:VS, :].T
    loss = np.float32(res.results[0]["loss"][0, 0])
    return logits, loss
